# revision 60
# baseline (speedup 1.0000x reference)
"""Trainium2 Bass kernel for nn_Attention_5514738008849.

Dense transformer attention block with axial rotary embeddings:
  x:(8,1024,1024) -> qkv -> rope(q,k) -> softmax(qk^T/sqrt(d)) v -> proj+bias

Sharding: pure data-parallel over batch B=8 across the 8 NeuronCores (one
batch element per core, full weights replicated). No collectives.

Per-core dataflow:
  - QKV GEMM in fp8e4 DoubleRow (0.5 cycles/row, 2 k-tiles of 128 per
    instruction) using a 3-term hi/lo split computed ON THE HOST:
      x ~ (x_hi + x_lo)/16,  w ~ (w_hi + w_lo)/256   (e4m3)
      W^T x ~ (w_hi x_hi + w_hi x_lo + w_lo x_hi) / 4096
    12 DR matmuls per 128x512 output tile vs 16 fp32r row-passes (0.75x),
    and half the weight DMA bytes.
  - rotary: pair-shuffle via a 128x128 signed-permutation fp32r matmul, then
    q_rot = q*cos + shuf*sin elementwise on DVE; Q/K PSUM evacuation on DVE
    with the 1/4096 descale folded in.
  - logits^T[k,q] per head in fp32r (K=64; fp8 gives no gain there); exp on
    ACT (scale=1/8), software-pipelined across head boundaries.
  - AV in fp32r with a ones-column appended to V => psum row 64 holds the
    softmax denominator; normalize on DVE producing 32*attn directly
    (scale folded into the reciprocal), then split attn into fp8 hi
    (DVE cast) + lo (gpsimd subtract) for the projection.
  - proj in fp8 DR 3-term (w_proj host-split); PSUM descale 1/8192 on the
    ACT evacuation. b_proj is added on the host after the gather.
"""

import os
import sys

sys.path.insert(0, "/opt/trn_rl_repo")

# This kernel needs the axon-tunneled NeuronCores. A JAX_PLATFORMS=cpu pin
# (used by some harnesses for the jax reference) would prevent the axon
# backend from registering; clearing it here is a no-op when jax has already
# initialized and restores device visibility when it hasn't.
if os.environ.get("JAX_PLATFORMS", "") not in ("", None):
    if "axon" not in os.environ["JAX_PLATFORMS"]:
        os.environ.pop("JAX_PLATFORMS", None)

import numpy as np
import ml_dtypes

import concourse.bass as bass
import concourse.bacc as bacc_mod
import concourse.mybir as mybir
from concourse.bass_utils import run_bass_kernel_spmd
from concourse.tile import TileContext

B, N, C = 8, 1024, 1024
H, D = 16, 64          # heads, head dim
ROT = 32               # rotary dims per head (head_dim // 2)
FH = FW = 32           # token grid for axial rope
NCORES = 8
F32 = mybir.dt.float32
F32R = mybir.dt.float32r
FP8 = mybir.dt.float8e4
BF16 = mybir.dt.bfloat16
E4 = ml_dtypes.float8_e4m3

S_X = 16.0             # fp8 scale on x
S_W = 256.0            # fp8 scale on w_qkv / w_proj
S_A = 32.0             # fp8 scale on attn output
QKV_DESCALE = 1.0 / (S_X * S_W)      # 1/4096
PROJ_DESCALE = 1.0 / (S_A * S_W)     # 1/8192

DR = mybir.MatmulPerfMode.DoubleRow


def _host_tables():
    """Rotary cos/sin in d-major (dim-on-partition) layout + shuffle matrix."""
    dim_r = D // 4                                    # 16
    base = np.linspace(1.0, (FH * FW) / 2.0, dim_r // 2) * np.pi   # (8,)

    def axis_freqs(n):
        pos = np.linspace(-1.0, 1.0, n)
        f = pos[:, None] * base[None, :]              # (n, 8)
        return np.repeat(f, 2, axis=-1)               # (n, 16)

    fH = np.broadcast_to(axis_freqs(FH)[:, None, :], (FH, FW, dim_r))
    fW = np.broadcast_to(axis_freqs(FW)[None, :, :], (FH, FW, dim_r))
    freqs = np.concatenate([fH, fW], axis=-1).reshape(N, ROT)      # (1024, 32)

    # d-major table for one 128-partition block = two heads:
    # rows 0-31 rot (head even), 32-63 pass, 64-95 rot (head odd), 96-127 pass
    cos_d = np.ones((128, N), np.float32)
    sin_d = np.zeros((128, N), np.float32)
    ct = np.cos(freqs).T.astype(np.float32)           # (32, 1024)
    st = np.sin(freqs).T.astype(np.float32)
    cos_d[0:32] = ct
    cos_d[64:96] = ct
    sin_d[0:32] = st
    sin_d[64:96] = st

    # signed permutation: shuf[2i] = -q[2i+1], shuf[2i+1] = q[2i] on rot rows
    pshuf = np.zeros((128, 128), np.float32)
    for off in (0, 64):
        for i in range(ROT // 2):
            r0, r1 = off + 2 * i, off + 2 * i + 1
            pshuf[r1, r0] = -1.0                      # out[r0] = -in[r1]
            pshuf[r0, r1] = 1.0                       # out[r1] = +in[r0]

    bf = ml_dtypes.bfloat16
    return cos_d.astype(bf), sin_d.astype(bf), pshuf.astype(bf)


def _split8(a, s):
    """a ~ (hi + lo)/s with hi, lo e4m3."""
    hi = (s * a).astype(E4)
    lo = (s * a - hi.astype(np.float32)).astype(E4)
    return hi, lo


def _pack_w(w8):
    """[1024 k, M] e4m3 -> [128 p, M//512 og, 4 j, 2 a, 512 m] with
    row (2j+a)*128+p, col og*512+m, so each og-slice is one contiguous
    4KB/partition DMA and DR k-tile pairs (a) are adjacent."""
    M = w8.shape[1]
    t = w8.reshape(4, 2, 128, M // 512, 512)          # j a p og m
    return np.ascontiguousarray(t.transpose(2, 3, 0, 1, 4))


def _pack_x(x8t):
    """[1024 k, 1024 t] e4m3 -> [128 p, 4 j, 2 a, 1024 t], row (2j+a)*128+p."""
    t = x8t.reshape(4, 2, 128, N)                     # j a p t
    return np.ascontiguousarray(t.transpose(2, 0, 1, 3))


def _build_program():
    nc = bacc_mod.Bacc()
    xth_h = nc.declare_dram_parameter("xt_hi", [128, 4, 2, N], FP8, isOutput=False)
    xtl_h = nc.declare_dram_parameter("xt_lo", [128, 4, 2, N], FP8, isOutput=False)
    wqh_h = nc.declare_dram_parameter("wq_hi", [128, 6, 4, 2, 512], FP8, isOutput=False)
    wql_h = nc.declare_dram_parameter("wq_lo", [128, 6, 4, 2, 512], FP8, isOutput=False)
    wph_h = nc.declare_dram_parameter("wp_hi", [128, 2, 4, 2, 512], FP8, isOutput=False)
    wpl_h = nc.declare_dram_parameter("wp_lo", [128, 2, 4, 2, 512], FP8, isOutput=False)
    cos_h = nc.declare_dram_parameter("cos_d", [128, N], BF16, isOutput=False)
    sin_h = nc.declare_dram_parameter("sin_d", [128, N], BF16, isOutput=False)
    pshuf_h = nc.declare_dram_parameter("pshuf", [128, 128], BF16, isOutput=False)
    onescol_h = nc.declare_dram_parameter("ones_col", [128, 64], F32, isOutput=False)
    out_h = nc.declare_dram_parameter("out", [N, C], F32, isOutput=True)

    def f32r(ap):
        return ap.bitcast(F32R)

    from contextlib import ExitStack
    with ExitStack() as top:
        top.enter_context(
            nc.allow_low_precision(reason="fp8e4 DoubleRow + fp32r operands"))
        tc = top.enter_context(TileContext(nc))
        consts = top.enter_context(tc.tile_pool(name="consts", bufs=1))
        xtp = top.enter_context(tc.tile_pool(name="xtp", bufs=1))
        vxp = [top.enter_context(tc.tile_pool(name=f"vx{i}", bufs=1))
               for i in range(2)]
        qkp = [top.enter_context(tc.tile_pool(name=f"qk{hp}", bufs=1))
               for hp in range(8)]
        cos_sb = consts.tile([128, N], BF16)
        sin_sb = consts.tile([128, N], BF16)
        pshuf_sb = consts.tile([128, 128], BF16)

        # persistent through phases 2-3; per-hp pools for Q/K so the
        # scheduler's pool-granular watermarks stay precise, and separate
        # pools for the two V halves (og5 is computed during attention)
        qrot_sb = [qkp[hp].tile([128, N], BF16, name=f"qr{hp}")
                   for hp in range(8)]
        krot_sb = [qkp[hp].tile([128, N], BF16, name=f"kr{hp}")
                   for hp in range(8)]
        vext_sb = [vxp[i].tile([128, 8, 8, 65], F32, name=f"vext{i}")
                   for i in range(2)]

        xhi_sb = xtp.tile([128, 4, 2, N], FP8)
        xlo_sb = xtp.tile([128, 4, 2, N], FP8)

        # ---- phase 1: stream x hi/lo on the SWDGE queue, parallel with
        # weights on HWDGE; per-j chunks, hi/lo interleaved (the SWDGE
        # drains ~1.7us per 256KB chunk serially, so chunk count is the
        # startup critical path) ----
        for j in range(4):
            nc.gpsimd.dma_start(out=xhi_sb[:, j, :, :],
                                in_=xth_h[:, j, :, :])
            nc.gpsimd.dma_start(out=xlo_sb[:, j, :, :],
                                in_=xtl_h[:, j, :, :])

        def v_chain(v_ps, tb, whi_t, wlo_t):
            idx = 0
            for j in range(4):
                for xt, wt in ((xhi_sb, whi_t), (xhi_sb, wlo_t),
                               (xlo_sb, whi_t)):
                    nc.tensor.matmul(
                        v_ps,
                        xt[:, j, :, tb * 128:(tb + 1) * 128],
                        wt[:, j, :, :],
                        start=(idx == 0),
                        stop=(idx == 11),
                        perf_mode=DR,
                    )
                    idx += 1

        def v_evac(v_ps, vh, tb):
            nc.vector.tensor_scalar_mul(
                f32r(vext_sb[vh][:, tb, :, 0:64]),
                v_ps.rearrange("p (a b) -> p a b", a=8),
                QKV_DESCALE,
            )

        # ---- phase 2: QKV (fp8 DR 3-term) + rotary + V_ext(half 0) ----
        with (
            tc.tile_pool(name="wq", bufs=4) as wq,
            tc.tile_pool(name="rot", bufs=3) as rot,
            tc.tile_pool(name="ps_qkv", bufs=3, space="PSUM") as ps_qkv,
            tc.tile_pool(name="ps_misc", bufs=1, space="PSUM") as ps_misc,
        ):
            for og in (4, 0, 2, 1, 3, 5):
                whi_t = wq.tile([128, 4, 2, 512], FP8, tag="w_t",
                                name=f"whi{og}")
                wlo_t = wq.tile([128, 4, 2, 512], FP8, tag="w_t",
                                name=f"wlo{og}")
                # per-j chunks: each dma_start's transfer runs on one DMA
                # engine, so splitting is what buys transfer parallelism
                for j in range(4):
                    nc.sync.dma_start(out=whi_t[:, j, :, :],
                                      in_=wqh_h[:, og, j, :, :])
                    nc.sync.dma_start(out=wlo_t[:, j, :, :],
                                      in_=wql_h[:, og, j, :, :])
                if og == 0:
                    # rotary tables: must be emitted before og0's rotary
                    # reads them (dep tracking follows emission order), but
                    # after og4+og0 weight chunks in the HWDGE queue
                    nc.sync.dma_start(out=cos_sb, in_=cos_h[:, :])
                    nc.sync.dma_start(out=sin_sb, in_=sin_h[:, :])
                    nc.sync.dma_start(out=pshuf_sb, in_=pshuf_h[:, :])
                if og == 3:
                    # vext ones columns (first AV needs them ~75us in)
                    for vh in range(2):
                        for tb in range(8):
                            nc.sync.dma_start(
                                out=f32r(vext_sb[vh][:, tb, :, 64:65]),
                                in_=f32r(onescol_h[:, 0:8]),
                            )

                if og < 4:                    # Q^T / K^T (d-major)
                    for jj in range(4):
                        ob = og * 4 + jj      # global 128-out block
                        qkv_ps = ps_qkv.tile([128, N], F32, tag="qkv_ps",
                                             name=f"qkv_ps{ob}")
                        for qc in range(2):
                            idx = 0
                            for j in range(4):
                                for wt, xt in ((whi_t, xhi_sb),
                                               (whi_t, xlo_sb),
                                               (wlo_t, xhi_sb)):
                                    nc.tensor.matmul(
                                        qkv_ps[:, qc * 512:(qc + 1) * 512],
                                        wt[:, j, :, jj * 128:(jj + 1) * 128],
                                        xt[:, j, :, qc * 512:(qc + 1) * 512],
                                        start=(idx == 0),
                                        stop=(idx == 11),
                                        perf_mode=DR,
                                    )
                                    idx += 1
                        hp = ob % 8
                        dst = (qrot_sb if ob < 8 else krot_sb)[hp]
                        q_sb = rot.tile([128, N], BF16, tag="q_sb")
                        nc.scalar.mul(q_sb, qkv_ps, QKV_DESCALE)
                        shuf_ps = ps_misc.tile([128, N], F32, tag="shuf_ps",
                                               name=f"shuf{ob}")
                        for qc in range(2):
                            nc.tensor.matmul(
                                shuf_ps[:, qc * 512:(qc + 1) * 512],
                                pshuf_sb,
                                q_sb[:, qc * 512:(qc + 1) * 512],
                                start=True,
                                stop=True,
                            )
                        tmp = rot.tile([128, N], BF16, tag="tmp")
                        nc.vector.tensor_mul(tmp, shuf_ps, sin_sb)
                        nc.vector.tensor_mul(dst, q_sb, cos_sb)
                        nc.vector.tensor_add(dst, dst, tmp)
                else:                         # V halves (token-major)
                    for tb in range(8):
                        v_ps = ps_qkv.tile([128, 512], F32, tag="qkv_ps",
                                           name=f"v_ps{og}_{tb}")
                        v_chain(v_ps, tb, whi_t, wlo_t)
                        v_evac(v_ps, og - 4, tb)

        # ============ phases 3-4 (attn scoped here) ============
        with tc.tile_pool(name="attnp0", bufs=1) as ap0, \
             tc.tile_pool(name="attnp1", bufs=1) as ap1, \
             tc.tile_pool(name="attnp2", bufs=1) as ap2, \
             tc.tile_pool(name="attnp3", bufs=1) as ap3:
            # 32*attn^T hi/lo (c-major), one POOL per proj j-pair (heads
            # 4j..4j+3): the scheduler's cross-engine watermarks are
            # pool-granular, so per-j pools keep early-head proj reads
            # from waiting on the last head's normalize
            apools = [ap0, ap1, ap2, ap3]
            ahi_sb = [apools[j].tile([128, 2, N], FP8, name=f"ahi{j}")
                      for j in range(4)]
            alo_sb = [apools[j].tile([128, 2, N], FP8, name=f"alo{j}")
                      for j in range(4)]

            # ---- phase 3: attention in fp32r, with og5's V chains
            # injected into the ACT-bound window (heads 0-7) ----
            with tc.tile_pool(name="wpre", bufs=4) as wpre:
                # w_proj prefetch for phase 4
                wp_tiles = []
                for og in range(2):
                    wph_t = wpre.tile([128, 4, 2, 512], FP8, tag="wp_t",
                                      name=f"wph{og}")
                    wpl_t = wpre.tile([128, 4, 2, 512], FP8, tag="wp_t",
                                      name=f"wpl{og}")
                    nc.sync.dma_start(out=wph_t, in_=wph_h[:, og, :, :, :])
                    nc.sync.dma_start(out=wpl_t, in_=wpl_h[:, og, :, :, :])
                    wp_tiles.append((wph_t, wpl_t))

                with (
                    tc.tile_pool(name="expp", bufs=4) as expp,
                    tc.tile_pool(name="navp", bufs=2) as navp,
                    tc.tile_pool(name="ps_lg", bufs=2, space="PSUM") as ps_lg,
                    tc.tile_pool(name="ps_av", bufs=2, space="PSUM") as ps_av,
                ):
                    def emit_logits(h, kt):
                        hp, r0 = h // 2, (h % 2) * 64
                        lg_ps = ps_lg.tile([128, N], F32, tag="lg_ps",
                                           name=f"lg{h}_{kt}")
                        lhs = krot_sb[hp][r0:r0 + 64,
                                          kt * 128:(kt + 1) * 128]
                        for qc in range(2):
                            nc.tensor.matmul(
                                lg_ps[:, qc * 512:(qc + 1) * 512],
                                lhs,
                                qrot_sb[hp][r0:r0 + 64,
                                            qc * 512:(qc + 1) * 512],
                                start=True,
                                stop=True,
                            )
                        return lg_ps

                    # Software pipeline depth 1 across ALL kt blocks: the
                    # next logits matmuls are emitted BEFORE this block's
                    # exp/AV, so the in-order PE SEQ computes lg(kt+1)
                    # while ACT runs exp(kt) instead of stalling on the
                    # AV(kt) semaphore — keeps the exp pipe gapless.
                    lg_cur = emit_logits(0, 0)
                    for h in range(H):
                        hp, r0 = h // 2, (h % 2) * 64
                        av_ps = ps_av.tile([65, N], F32, tag="av_ps",
                                           name=f"av{h}")
                        for kt in range(8):
                            if kt < 7:
                                lg_nxt = emit_logits(h, kt + 1)
                            elif h + 1 < H:
                                lg_nxt = emit_logits(h + 1, 0)
                            else:
                                lg_nxt = None
                            e_sb = expp.tile([128, N], F32, tag="e_sb",
                                             name=f"e{h}_{kt}")
                            nc.scalar.activation(
                                f32r(e_sb), lg_cur,
                                mybir.ActivationFunctionType.Exp, scale=0.125,
                            )
                            for qc in range(2):
                                nc.tensor.matmul(
                                    av_ps[:, qc * 512:(qc + 1) * 512],
                                    f32r(vext_sb[h // 8][:, kt, h % 8, :]),
                                    f32r(e_sb[:, qc * 512:(qc + 1) * 512]),
                                    start=(kt == 0),
                                    stop=(kt == 7),
                                )
                            lg_cur = lg_nxt
                        # normalize: the V ones-column is 1/S_A, so the psum
                        # denominator row is den/S_A and rb = S_A/den comes
                        # straight out of the reciprocal; attn32 = S_A*attn
                        recip = navp.tile([1, N], F32, tag="recip", bufs=1)
                        nc.vector.reciprocal(f32r(recip), av_ps[64:65, :])
                        rb = navp.tile([64, N], F32, tag="rb_sb", bufs=1)
                        nc.gpsimd.partition_broadcast(rb, recip)
                        attn32_t = navp.tile([128, N], F32, tag="attn32")
                        attn32 = attn32_t[r0:r0 + 64, :]
                        nc.vector.tensor_mul(attn32, av_ps[0:64, :], rb)
                        ahi = ahi_sb[hp // 2][r0:r0 + 64, hp % 2, :]
                        nc.vector.tensor_copy(ahi, attn32)
                        alo = alo_sb[hp // 2][r0:r0 + 64, hp % 2, :]
                        if h >= H - 4:
                            # j3 heads gate the proj tail: DVE sub is the
                            # shorter path than the Pool sub
                            nc.vector.tensor_sub(alo, attn32, ahi)
                        else:
                            nc.gpsimd.tensor_sub(alo, attn32, ahi)

                    # ---- phase 4: proj (fp8 DR 3-term) ----
                    # y tiles reuse the lg_ps ring (same pool+tag): those
                    # slots free right after the last exp, so proj is not
                    # gated on the last head's normalize chain the way a
                    # fresh PSUM pool (bank handover barrier) would be.
                    with tc.tile_pool(name="yout", bufs=4) as yout:
                        # Staggered emission: tb's j3 terms (which need the
                        # last heads' fp8 split) are emitted after tb+1's
                        # j0-2 terms, so the PE has ~2us of runway while
                        # the final normalize drains.  y tiles reuse the
                        # lg_ps ring (bufs=2 -> tb and tb+1 in flight).
                        def part_a(y_ps, tb):
                            for oc in range(2):
                                wph_t, wpl_t = wp_tiles[oc]
                                idx = 0
                                for j in range(3):
                                    for at, wt in ((ahi_sb[j], wph_t),
                                                   (ahi_sb[j], wpl_t),
                                                   (alo_sb[j], wph_t)):
                                        nc.tensor.matmul(
                                            y_ps[:, oc * 512:(oc + 1) * 512],
                                            at[:, :, tb * 128:(tb + 1) * 128],
                                            wt[:, j, :, :],
                                            start=(idx == 0),
                                            stop=False,
                                            perf_mode=DR,
                                        )
                                        idx += 1

                        def part_b(y_ps, tb):
                            for oc in range(2):
                                wph_t, wpl_t = wp_tiles[oc]
                                for i, (at, wt) in enumerate(
                                        ((ahi_sb[3], wph_t), (ahi_sb[3], wpl_t),
                                         (alo_sb[3], wph_t))):
                                    nc.tensor.matmul(
                                        y_ps[:, oc * 512:(oc + 1) * 512],
                                        at[:, :, tb * 128:(tb + 1) * 128],
                                        wt[:, 3, :, :],
                                        start=False,
                                        stop=(i == 2),
                                        perf_mode=DR,
                                    )
                                y_sb = yout.tile([128, 512], F32, tag="y_sb",
                                                 name=f"y_sb{tb}_{oc}")
                                nc.scalar.mul(
                                    y_sb, y_ps[:, oc * 512:(oc + 1) * 512],
                                    PROJ_DESCALE)
                                nc.sync.dma_start(
                                    out=out_h[tb * 128:(tb + 1) * 128,
                                              oc * 512:(oc + 1) * 512],
                                    in_=y_sb,
                                )

                        y_tiles = [None] * 8
                        for tb in range(8):
                            if tb >= 2:
                                part_b(y_tiles[tb - 2], tb - 2)
                            y_tiles[tb] = ps_lg.tile([128, C], F32,
                                                     tag="lg_ps",
                                                     name=f"y_ps{tb}")
                            part_a(y_tiles[tb], tb)
                        part_b(y_tiles[6], 6)
                        part_b(y_tiles[7], 7)
    nc.finalize()
    return nc


_PROGRAM = None
_TABLES = None


def kernel(x, w_qkv, w_proj, b_proj):
    global _PROGRAM, _TABLES
    if _PROGRAM is None:
        _PROGRAM = _build_program()
    nc = _PROGRAM

    if _TABLES is None:
        _TABLES = _host_tables()
    cos_d, sin_d, pshuf = _TABLES
    wqh, wql = _split8(np.asarray(w_qkv, np.float32), S_W)
    wph, wpl = _split8(np.asarray(w_proj, np.float32), S_W)
    shared = {
        "wq_hi": _pack_w(wqh),
        "wq_lo": _pack_w(wql),
        "wp_hi": _pack_w(wph),
        "wp_lo": _pack_w(wpl),
        "cos_d": cos_d,
        "sin_d": sin_d,
        "pshuf": pshuf,
        "ones_col": np.full((128, 64), 1.0 / S_A, np.float32),
    }

    in_maps = []
    for b in range(NCORES):
        xt = np.asarray(x[b], np.float32).T          # [C, N]
        xhi, xlo = _split8(xt, S_X)
        in_maps.append({"xt_hi": _pack_x(xhi), "xt_lo": _pack_x(xlo), **shared})

    res = run_bass_kernel_spmd(nc, in_maps, core_ids=list(range(NCORES)))
    bias = np.asarray(b_proj, np.float32)[None, :]
    return np.stack([res.results[b]["out"] + bias for b in range(NCORES)],
                    axis=0)


if __name__ == "__main__":
    xs = np.random.randn(B, N, C).astype(np.float32)
    wq = (np.random.randn(C, 3 * C) / np.sqrt(C)).astype(np.float32)
    wp = (np.random.randn(C, C) / np.sqrt(C)).astype(np.float32)
    bp = (np.random.randn(C) * 0.01).astype(np.float32)
    out = kernel(x=xs, w_qkv=wq, w_proj=wp, b_proj=bp)
    print(out.shape, out.dtype)


# revision 66
# speedup vs baseline: 1.0188x; 1.0188x over previous
"""Trainium2 Bass kernel for nn_Attention_5514738008849.

Dense transformer attention block with axial rotary embeddings:
  x:(8,1024,1024) -> qkv -> rope(q,k) -> softmax(qk^T/sqrt(d)) v -> proj+bias

Sharding: pure data-parallel over batch B=8 across the 8 NeuronCores (one
batch element per core, full weights replicated). No collectives.

Per-core dataflow:
  - QKV GEMM in fp8e4 DoubleRow (0.5 cycles/row, 2 k-tiles of 128 per
    instruction) using a 3-term hi/lo split computed ON THE HOST:
      x ~ (x_hi + x_lo)/16,  w ~ (w_hi + w_lo)/256   (e4m3)
      W^T x ~ (w_hi x_hi + w_hi x_lo + w_lo x_hi) / 4096
    12 DR matmuls per 128x512 output tile vs 16 fp32r row-passes (0.75x),
    and half the weight DMA bytes.
  - rotary: pair-shuffle via a 128x128 signed-permutation fp32r matmul, then
    q_rot = q*cos + shuf*sin elementwise on DVE; Q/K PSUM evacuation on DVE
    with the 1/4096 descale folded in.
  - logits^T[k,q] per head in fp32r (K=64; fp8 gives no gain there); exp on
    ACT (scale=1/8), software-pipelined across head boundaries.
  - AV in fp32r with a ones-column appended to V => psum row 64 holds the
    softmax denominator; normalize on DVE producing 32*attn directly
    (scale folded into the reciprocal), then split attn into fp8 hi
    (DVE cast) + lo (gpsimd subtract) for the projection.
  - proj in fp8 DR 3-term (w_proj host-split); PSUM descale 1/8192 on the
    ACT evacuation. b_proj is added on the host after the gather.
"""

import os
import sys

sys.path.insert(0, "/opt/trn_rl_repo")

# This kernel needs the axon-tunneled NeuronCores. A JAX_PLATFORMS=cpu pin
# (used by some harnesses for the jax reference) would prevent the axon
# backend from registering; clearing it here is a no-op when jax has already
# initialized and restores device visibility when it hasn't.
if os.environ.get("JAX_PLATFORMS", "") not in ("", None):
    if "axon" not in os.environ["JAX_PLATFORMS"]:
        os.environ.pop("JAX_PLATFORMS", None)

import numpy as np
import ml_dtypes

import concourse.bass as bass
import concourse.bacc as bacc_mod
import concourse.mybir as mybir
from concourse.bass_utils import run_bass_kernel_spmd
from concourse.tile import TileContext

B, N, C = 8, 1024, 1024
H, D = 16, 64          # heads, head dim
ROT = 32               # rotary dims per head (head_dim // 2)
FH = FW = 32           # token grid for axial rope
NCORES = 8
F32 = mybir.dt.float32
F32R = mybir.dt.float32r
FP8 = mybir.dt.float8e4
BF16 = mybir.dt.bfloat16
E4 = ml_dtypes.float8_e4m3

S_X = 16.0             # fp8 scale on x
S_W = 256.0            # fp8 scale on w_qkv / w_proj
S_A = 32.0             # fp8 scale on attn output
QKV_DESCALE = 1.0 / (S_X * S_W)      # 1/4096
PROJ_DESCALE = 1.0 / (S_A * S_W)     # 1/8192

DR = mybir.MatmulPerfMode.DoubleRow


def _host_tables():
    """Rotary cos/sin in d-major (dim-on-partition) layout + shuffle matrix."""
    dim_r = D // 4                                    # 16
    base = np.linspace(1.0, (FH * FW) / 2.0, dim_r // 2) * np.pi   # (8,)

    def axis_freqs(n):
        pos = np.linspace(-1.0, 1.0, n)
        f = pos[:, None] * base[None, :]              # (n, 8)
        return np.repeat(f, 2, axis=-1)               # (n, 16)

    fH = np.broadcast_to(axis_freqs(FH)[:, None, :], (FH, FW, dim_r))
    fW = np.broadcast_to(axis_freqs(FW)[None, :, :], (FH, FW, dim_r))
    freqs = np.concatenate([fH, fW], axis=-1).reshape(N, ROT)      # (1024, 32)

    # d-major table for one 128-partition block = two heads:
    # rows 0-31 rot (head even), 32-63 pass, 64-95 rot (head odd), 96-127 pass
    cos_d = np.ones((128, N), np.float32)
    sin_d = np.zeros((128, N), np.float32)
    ct = np.cos(freqs).T.astype(np.float32)           # (32, 1024)
    st = np.sin(freqs).T.astype(np.float32)
    cos_d[0:32] = ct
    cos_d[64:96] = ct
    sin_d[0:32] = st
    sin_d[64:96] = st

    # signed permutation: shuf[2i] = -q[2i+1], shuf[2i+1] = q[2i] on rot rows
    pshuf = np.zeros((128, 128), np.float32)
    for off in (0, 64):
        for i in range(ROT // 2):
            r0, r1 = off + 2 * i, off + 2 * i + 1
            pshuf[r1, r0] = -1.0                      # out[r0] = -in[r1]
            pshuf[r0, r1] = 1.0                       # out[r1] = +in[r0]

    bf = ml_dtypes.bfloat16
    return cos_d.astype(bf), sin_d.astype(bf), pshuf.astype(bf)


def _split8(a, s):
    """a ~ (hi + lo)/s with hi, lo e4m3."""
    hi = (s * a).astype(E4)
    lo = (s * a - hi.astype(np.float32)).astype(E4)
    return hi, lo


def _pack_w(w8):
    """[1024 k, M] e4m3 -> [128 p, M//512 og, 4 j, 2 a, 512 m] with
    row (2j+a)*128+p, col og*512+m, so each og-slice is one contiguous
    4KB/partition DMA and DR k-tile pairs (a) are adjacent."""
    M = w8.shape[1]
    t = w8.reshape(4, 2, 128, M // 512, 512)          # j a p og m
    return np.ascontiguousarray(t.transpose(2, 3, 0, 1, 4))


def _pack_x(x8t):
    """[1024 k, 1024 t] e4m3 -> [128 p, 4 j, 2 a, 1024 t], row (2j+a)*128+p."""
    t = x8t.reshape(4, 2, 128, N)                     # j a p t
    return np.ascontiguousarray(t.transpose(2, 0, 1, 3))


def _build_program():
    nc = bacc_mod.Bacc()
    xth_h = nc.declare_dram_parameter("xt_hi", [128, 4, 2, N], FP8, isOutput=False)
    xtl_h = nc.declare_dram_parameter("xt_lo", [128, 4, 2, N], FP8, isOutput=False)
    wqh_h = nc.declare_dram_parameter("wq_hi", [128, 6, 4, 2, 512], FP8, isOutput=False)
    wql_h = nc.declare_dram_parameter("wq_lo", [128, 6, 4, 2, 512], FP8, isOutput=False)
    wph_h = nc.declare_dram_parameter("wp_hi", [128, 2, 4, 2, 512], FP8, isOutput=False)
    wpl_h = nc.declare_dram_parameter("wp_lo", [128, 2, 4, 2, 512], FP8, isOutput=False)
    cos_h = nc.declare_dram_parameter("cos_d", [128, N], BF16, isOutput=False)
    sin_h = nc.declare_dram_parameter("sin_d", [128, N], BF16, isOutput=False)
    pshuf_h = nc.declare_dram_parameter("pshuf", [128, 128], BF16, isOutput=False)
    onescol_h = nc.declare_dram_parameter("ones_col", [128, 64], F32, isOutput=False)
    out_h = nc.declare_dram_parameter("out", [N, C], F32, isOutput=True)

    def f32r(ap):
        return ap.bitcast(F32R)

    from contextlib import ExitStack
    with ExitStack() as top:
        top.enter_context(
            nc.allow_low_precision(reason="fp8e4 DoubleRow + fp32r operands"))
        tc = top.enter_context(TileContext(nc))
        consts = top.enter_context(tc.tile_pool(name="consts", bufs=1))
        xtp = top.enter_context(tc.tile_pool(name="xtp", bufs=1))
        vxp = [top.enter_context(tc.tile_pool(name=f"vx{i}", bufs=1))
               for i in range(2)]
        qkp = [top.enter_context(tc.tile_pool(name=f"qk{hp}", bufs=1))
               for hp in range(8)]
        cos_sb = consts.tile([128, N], BF16)
        sin_sb = consts.tile([128, N], BF16)
        pshuf_sb = consts.tile([128, 128], BF16)

        # persistent through phases 2-3; per-hp pools for Q/K so the
        # scheduler's pool-granular watermarks stay precise, and separate
        # pools for the two V halves (og5 is computed during attention)
        qrot_sb = [qkp[hp].tile([128, N], BF16, name=f"qr{hp}")
                   for hp in range(8)]
        krot_sb = [qkp[hp].tile([128, N], BF16, name=f"kr{hp}")
                   for hp in range(8)]
        vext_sb = [vxp[i].tile([128, 8, 8, 65], F32, name=f"vext{i}")
                   for i in range(2)]

        xhi_sb = xtp.tile([128, 4, 2, N], FP8)
        xlo_sb = xtp.tile([128, 4, 2, N], FP8)

        # ---- phase 1: stream x hi/lo on the SWDGE queue, parallel with
        # weights on HWDGE; per-j chunks, hi/lo interleaved (the SWDGE
        # drains ~1.7us per 256KB chunk serially, so chunk count is the
        # startup critical path) ----
        for j in range(4):
            nc.gpsimd.dma_start(out=xhi_sb[:, j, :, :],
                                in_=xth_h[:, j, :, :])
            nc.gpsimd.dma_start(out=xlo_sb[:, j, :, :],
                                in_=xtl_h[:, j, :, :])

        def v_chain(v_ps, tb, whi_t, wlo_t):
            idx = 0
            for j in range(4):
                for xt, wt in ((xhi_sb, whi_t), (xhi_sb, wlo_t),
                               (xlo_sb, whi_t)):
                    nc.tensor.matmul(
                        v_ps,
                        xt[:, j, :, tb * 128:(tb + 1) * 128],
                        wt[:, j, :, :],
                        start=(idx == 0),
                        stop=(idx == 11),
                        perf_mode=DR,
                    )
                    idx += 1

        def v_evac(v_ps, vh, tb):
            nc.vector.tensor_scalar_mul(
                f32r(vext_sb[vh][:, tb, :, 0:64]),
                v_ps.rearrange("p (a b) -> p a b", a=8),
                QKV_DESCALE,
            )

        # ---- phase 2: QKV (fp8 DR 3-term) + rotary + V_ext(half 0) ----
        with (
            tc.tile_pool(name="wq", bufs=4) as wq,
            tc.tile_pool(name="rot", bufs=3) as rot,
            tc.tile_pool(name="ps_qkv", bufs=3, space="PSUM") as ps_qkv,
            tc.tile_pool(name="ps_misc", bufs=1, space="PSUM") as ps_misc,
        ):
            for og in (4, 0, 2, 1, 3, 5):
                whi_t = wq.tile([128, 4, 2, 512], FP8, tag="w_t",
                                name=f"whi{og}")
                wlo_t = wq.tile([128, 4, 2, 512], FP8, tag="w_t",
                                name=f"wlo{og}")
                # per-j chunks: each dma_start's transfer runs on one DMA
                # engine, so splitting is what buys transfer parallelism
                for j in range(4):
                    nc.sync.dma_start(out=whi_t[:, j, :, :],
                                      in_=wqh_h[:, og, j, :, :])
                    nc.sync.dma_start(out=wlo_t[:, j, :, :],
                                      in_=wql_h[:, og, j, :, :])
                if og == 0:
                    # rotary tables: must be emitted before og0's rotary
                    # reads them (dep tracking follows emission order), but
                    # after og4+og0 weight chunks in the HWDGE queue
                    nc.sync.dma_start(out=cos_sb, in_=cos_h[:, :])
                    nc.sync.dma_start(out=sin_sb, in_=sin_h[:, :])
                    nc.sync.dma_start(out=pshuf_sb, in_=pshuf_h[:, :])
                if og == 3:
                    # vext ones columns (first AV needs them ~75us in)
                    for vh in range(2):
                        for tb in range(8):
                            nc.sync.dma_start(
                                out=f32r(vext_sb[vh][:, tb, :, 64:65]),
                                in_=f32r(onescol_h[:, 0:8]),
                            )

                if og < 4:                    # Q^T / K^T (d-major)
                    for jj in range(4):
                        ob = og * 4 + jj      # global 128-out block
                        qkv_ps = ps_qkv.tile([128, N], F32, tag="qkv_ps",
                                             name=f"qkv_ps{ob}")
                        for qc in range(2):
                            idx = 0
                            for j in range(4):
                                for wt, xt in ((whi_t, xhi_sb),
                                               (whi_t, xlo_sb),
                                               (wlo_t, xhi_sb)):
                                    nc.tensor.matmul(
                                        qkv_ps[:, qc * 512:(qc + 1) * 512],
                                        wt[:, j, :, jj * 128:(jj + 1) * 128],
                                        xt[:, j, :, qc * 512:(qc + 1) * 512],
                                        start=(idx == 0),
                                        stop=(idx == 11),
                                        perf_mode=DR,
                                    )
                                    idx += 1
                        hp = ob % 8
                        dst = (qrot_sb if ob < 8 else krot_sb)[hp]
                        q_sb = rot.tile([128, N], BF16, tag="q_sb")
                        nc.scalar.mul(q_sb, qkv_ps, QKV_DESCALE)
                        shuf_ps = ps_misc.tile([128, N], F32, tag="shuf_ps",
                                               name=f"shuf{ob}")
                        for qc in range(2):
                            nc.tensor.matmul(
                                shuf_ps[:, qc * 512:(qc + 1) * 512],
                                pshuf_sb,
                                q_sb[:, qc * 512:(qc + 1) * 512],
                                start=True,
                                stop=True,
                            )
                        tmp = rot.tile([128, N], BF16, tag="tmp")
                        nc.vector.tensor_mul(tmp, shuf_ps, sin_sb)
                        nc.vector.tensor_mul(dst, q_sb, cos_sb)
                        nc.vector.tensor_add(dst, dst, tmp)
                else:                         # V halves (token-major)
                    for tb in range(8):
                        v_ps = ps_qkv.tile([128, 512], F32, tag="qkv_ps",
                                           name=f"v_ps{og}_{tb}")
                        v_chain(v_ps, tb, whi_t, wlo_t)
                        v_evac(v_ps, og - 4, tb)

        # ============ phases 3-4 (attn scoped here) ============
        with tc.tile_pool(name="attnp0", bufs=1) as ap0, \
             tc.tile_pool(name="attnp1", bufs=1) as ap1, \
             tc.tile_pool(name="attnp2", bufs=1) as ap2, \
             tc.tile_pool(name="attnp3", bufs=1) as ap3:
            # 32*attn^T hi/lo (c-major), one POOL per proj j-pair (heads
            # 4j..4j+3): the scheduler's cross-engine watermarks are
            # pool-granular, so per-j pools keep early-head proj reads
            # from waiting on the last head's normalize
            apools = [ap0, ap1, ap2, ap3]
            ahi_sb = [apools[j].tile([128, 2, N], FP8, name=f"ahi{j}")
                      for j in range(4)]
            alo_sb = [apools[j].tile([128, 2, N], FP8, name=f"alo{j}")
                      for j in range(4)]

            # ---- phase 3: attention in fp32r, with og5's V chains
            # injected into the ACT-bound window (heads 0-7) ----
            with tc.tile_pool(name="wpre", bufs=4) as wpre:
                # w_proj prefetch for phase 4
                wp_tiles = []
                for og in range(2):
                    wph_t = wpre.tile([128, 4, 2, 512], FP8, tag="wp_t",
                                      name=f"wph{og}")
                    wpl_t = wpre.tile([128, 4, 2, 512], FP8, tag="wp_t",
                                      name=f"wpl{og}")
                    nc.sync.dma_start(out=wph_t, in_=wph_h[:, og, :, :, :])
                    nc.sync.dma_start(out=wpl_t, in_=wpl_h[:, og, :, :, :])
                    wp_tiles.append((wph_t, wpl_t))

                with (
                    tc.tile_pool(name="expp", bufs=4) as expp,
                    tc.tile_pool(name="navp", bufs=2) as navp,
                    tc.tile_pool(name="ps_lg", bufs=2, space="PSUM") as ps_lg,
                    tc.tile_pool(name="ps_av", bufs=2, space="PSUM") as ps_av,
                ):
                    def emit_logits(h, kt):
                        hp, r0 = h // 2, (h % 2) * 64
                        lg_ps = ps_lg.tile([128, N], F32, tag="lg_ps",
                                           name=f"lg{h}_{kt}")
                        lhs = krot_sb[hp][r0:r0 + 64,
                                          kt * 128:(kt + 1) * 128]
                        for qc in range(2):
                            nc.tensor.matmul(
                                lg_ps[:, qc * 512:(qc + 1) * 512],
                                lhs,
                                qrot_sb[hp][r0:r0 + 64,
                                            qc * 512:(qc + 1) * 512],
                                start=True,
                                stop=True,
                            )
                        return lg_ps

                    # Software pipeline depth 1 across ALL kt blocks: the
                    # next logits matmuls are emitted BEFORE this block's
                    # exp/AV, so the in-order PE SEQ computes lg(kt+1)
                    # while ACT runs exp(kt) instead of stalling on the
                    # AV(kt) semaphore — keeps the exp pipe gapless.
                    lg_cur = emit_logits(0, 0)
                    for h in range(H):
                        hp, r0 = h // 2, (h % 2) * 64
                        av_ps = ps_av.tile([65, N], F32, tag="av_ps",
                                           name=f"av{h}")
                        for kt in range(8):
                            if kt < 7:
                                lg_nxt = emit_logits(h, kt + 1)
                            elif h + 1 < H:
                                lg_nxt = emit_logits(h + 1, 0)
                            else:
                                lg_nxt = None
                            e_sb = expp.tile([128, N], F32, tag="e_sb",
                                             name=f"e{h}_{kt}")
                            nc.scalar.activation(
                                f32r(e_sb), lg_cur,
                                mybir.ActivationFunctionType.Exp, scale=0.125,
                            )
                            for qc in range(2):
                                nc.tensor.matmul(
                                    av_ps[:, qc * 512:(qc + 1) * 512],
                                    f32r(vext_sb[h // 8][:, kt, h % 8, :]),
                                    f32r(e_sb[:, qc * 512:(qc + 1) * 512]),
                                    start=(kt == 0),
                                    stop=(kt == 7),
                                )
                            lg_cur = lg_nxt
                        # normalize: the V ones-column is 1/S_A, so the psum
                        # denominator row is den/S_A and rb = S_A/den comes
                        # straight out of the reciprocal; attn32 = S_A*attn
                        recip = navp.tile([1, N], F32, tag="recip", bufs=1)
                        nc.vector.reciprocal(f32r(recip), av_ps[64:65, :])
                        rb = navp.tile([64, N], F32, tag="rb_sb", bufs=1)
                        nc.gpsimd.partition_broadcast(rb, recip)
                        attn32_t = navp.tile([128, N], F32, tag="attn32")
                        attn32 = attn32_t[r0:r0 + 64, :]
                        nc.vector.tensor_mul(attn32, av_ps[0:64, :], rb)
                        ahi = ahi_sb[hp // 2][r0:r0 + 64, hp % 2, :]
                        nc.vector.tensor_copy(ahi, attn32)
                        alo = alo_sb[hp // 2][r0:r0 + 64, hp % 2, :]
                        if h >= H - 4:
                            # j3 heads gate the proj tail: DVE sub is the
                            # shorter path than the Pool sub
                            nc.vector.tensor_sub(alo, attn32, ahi)
                        else:
                            nc.gpsimd.tensor_sub(alo, attn32, ahi)

                    # ---- phase 4: proj (fp8 DR 3-term) ----
                    # y tiles reuse the lg_ps ring (same pool+tag): those
                    # slots free right after the last exp, so proj is not
                    # gated on the last head's normalize chain the way a
                    # fresh PSUM pool (bank handover barrier) would be.
                    with tc.tile_pool(name="yout", bufs=4) as yout:
                        # Staggered emission: tb's j3 terms (which need the
                        # last heads' fp8 split) are emitted after tb+1's
                        # j0-2 terms, so the PE has ~2us of runway while
                        # the final normalize drains.  y tiles reuse the
                        # lg_ps ring (bufs=2 -> tb and tb+1 in flight).
                        def part_a(y_ps, tb):
                            for oc in range(2):
                                wph_t, wpl_t = wp_tiles[oc]
                                idx = 0
                                for j in range(3):
                                    for at, wt in ((ahi_sb[j], wph_t),
                                                   (ahi_sb[j], wpl_t),
                                                   (alo_sb[j], wph_t)):
                                        nc.tensor.matmul(
                                            y_ps[:, oc * 512:(oc + 1) * 512],
                                            at[:, :, tb * 128:(tb + 1) * 128],
                                            wt[:, j, :, :],
                                            start=(idx == 0),
                                            stop=False,
                                            perf_mode=DR,
                                        )
                                        idx += 1

                        def part_b(y_ps, tb):
                            for oc in range(2):
                                wph_t, wpl_t = wp_tiles[oc]
                                for i, (at, wt) in enumerate(
                                        ((ahi_sb[3], wph_t), (ahi_sb[3], wpl_t),
                                         (alo_sb[3], wph_t))):
                                    nc.tensor.matmul(
                                        y_ps[:, oc * 512:(oc + 1) * 512],
                                        at[:, :, tb * 128:(tb + 1) * 128],
                                        wt[:, 3, :, :],
                                        start=False,
                                        stop=(i == 2),
                                        perf_mode=DR,
                                    )
                                y_sb = yout.tile([128, 512], F32, tag="y_sb",
                                                 name=f"y_sb{tb}_{oc}")
                                nc.scalar.mul(
                                    y_sb, y_ps[:, oc * 512:(oc + 1) * 512],
                                    PROJ_DESCALE)
                                nc.sync.dma_start(
                                    out=out_h[tb * 128:(tb + 1) * 128,
                                              oc * 512:(oc + 1) * 512],
                                    in_=y_sb,
                                )

                        # depth-3 stagger: y tiles alternate between the
                        # lg and av rings (the av ring is idle during proj),
                        # allowing 3 chains of runway before the first j3
                        # terms -- fully hiding the last-head normalize
                        y_tiles = [None] * 8
                        for tb in range(8):
                            if tb >= 3:
                                part_b(y_tiles[tb - 3], tb - 3)
                            pool = ps_lg if tb % 2 == 0 else ps_av
                            tag = "lg_ps" if tb % 2 == 0 else "av_ps"
                            y_tiles[tb] = pool.tile([128, C], F32, tag=tag,
                                                    name=f"y_ps{tb}")
                            part_a(y_tiles[tb], tb)
                        for tb in (5, 6, 7):
                            part_b(y_tiles[tb], tb)
    nc.finalize()
    return nc


_PROGRAM = None
_TABLES = None


def kernel(x, w_qkv, w_proj, b_proj):
    global _PROGRAM, _TABLES
    if _PROGRAM is None:
        _PROGRAM = _build_program()
    nc = _PROGRAM

    if _TABLES is None:
        _TABLES = _host_tables()
    cos_d, sin_d, pshuf = _TABLES
    wqh, wql = _split8(np.asarray(w_qkv, np.float32), S_W)
    wph, wpl = _split8(np.asarray(w_proj, np.float32), S_W)
    shared = {
        "wq_hi": _pack_w(wqh),
        "wq_lo": _pack_w(wql),
        "wp_hi": _pack_w(wph),
        "wp_lo": _pack_w(wpl),
        "cos_d": cos_d,
        "sin_d": sin_d,
        "pshuf": pshuf,
        "ones_col": np.full((128, 64), 1.0 / S_A, np.float32),
    }

    in_maps = []
    for b in range(NCORES):
        xt = np.asarray(x[b], np.float32).T          # [C, N]
        xhi, xlo = _split8(xt, S_X)
        in_maps.append({"xt_hi": _pack_x(xhi), "xt_lo": _pack_x(xlo), **shared})

    res = run_bass_kernel_spmd(nc, in_maps, core_ids=list(range(NCORES)))
    bias = np.asarray(b_proj, np.float32)[None, :]
    return np.stack([res.results[b]["out"] + bias for b in range(NCORES)],
                    axis=0)


if __name__ == "__main__":
    xs = np.random.randn(B, N, C).astype(np.float32)
    wq = (np.random.randn(C, 3 * C) / np.sqrt(C)).astype(np.float32)
    wp = (np.random.randn(C, C) / np.sqrt(C)).astype(np.float32)
    bp = (np.random.randn(C) * 0.01).astype(np.float32)
    out = kernel(x=xs, w_qkv=wq, w_proj=wp, b_proj=bp)
    print(out.shape, out.dtype)
